# revision 12
# baseline (speedup 1.0000x reference)
"""Trainium2 Bass kernel for DigitConvolutionalModel.

Computes: out = relu(conv2d_valid(x.reshape(B,28,28), w3x3)).reshape(B,676) @ W + b

Strategy (pure data parallel over 8 NeuronCores, 8192 images/core):
  - Host: swizzle x per core to xTs[112, 16 blocks, 7 chunks, 512 batch] bf16 so
    each block's DMA reads one 7KB-contiguous segment per partition; build two
    small constant matrices CA[112,104], CB[56,104] that express the 3x3 valid
    conv as matmuls over 4-image-row input chunks; pack W into 6 groups of 128
    features.
  - Device per core: stream 512-image blocks. Each block:
      x DMA alternates between the SP and DVE HWDGE rings (2x queue bw); the
      first block is split into per-chunk DMAs so the conv starts ~4us earlier.
      conv: 7 output groups; groups 0..5 = 2 accumulating PE matmuls
        (CA against chunk t, CB against first 2 rows of chunk t+1), group 6 =
        1 matmul. PSUM [104, 512] x7.
      ReLU PSUM->SBUF repacks the 676 features into 6 tiles of 128 partitions
        (12 partition-shifted copies spread over ACT/DVE/GPSIMD), so the fc
        needs only 6 accumulating matmuls (vs 7 for 104-feature groups).
      fc: 6 accumulating PE matmuls -> PSUM [10, 512]; bias-add via DVE;
        1 DMA store of outT [10, 512] on the GPSIMD ring.
  - Host: gather per-core outT [10, 8192] and transpose into out[B, 10].

Matmul dtype: bfloat16 (fp32 PSUM accumulation). Total HBM read per core is
12.9 MB; PE work is 19 passes x 512 cols x 16 blocks ~= 65us at 2.4 GHz.
"""

import os

import numpy as np

import concourse.bass as bass
import concourse.mybir as mybir
import concourse.tile as tile
from concourse import bacc
from concourse.bass import ts
from concourse.bass_utils import run_bass_kernel_spmd

# Problem geometry (fixed by the task spec)
B_FULL = 65536
IMG = 28
KW = 3
OH = IMG - KW + 1          # 26
NPIX = IMG * IMG           # 784
NFEAT = OH * OH            # 676
NOUT = 10

N_CORES = 8
B_CORE = B_FULL // N_CORES  # 8192
NB = 512                    # images per block
N_BLOCKS = B_CORE // NB     # 16

# Conv grouping: output rows in groups of 4 -> 104 features per group
G_ROWS = 4
G_FEAT = G_ROWS * OH        # 104
N_GROUPS = 7                # 6 full groups of 4 out-rows + 1 group of 2 (52 feats)
CHUNK_ROWS = 4              # input rows per SBUF partition chunk
CHUNK_PIX = CHUNK_ROWS * IMG  # 112
N_CHUNKS = NPIX // CHUNK_PIX  # 7

# fc grouping: features packed 128 per group
FC_K = 128
N_FC = (NFEAT + FC_K - 1) // FC_K  # 6 (5 x 128 + 36)

PRECISION = os.environ.get("KERNEL_PRECISION", "bf16")  # bf16 only


def _feat_count(t: int) -> int:
    # conv group t covers output rows [4t, min(4t+4, 26)) -> features [104t, ..)
    return min(G_FEAT, NFEAT - G_FEAT * t)


def _fc_count(k: int) -> int:
    return min(FC_K, NFEAT - FC_K * k)


def build_conv_mats(conv_w: np.ndarray):
    """CA[(ri,c),(ol,oj)] and CB[(ri2,c),(ol,oj)] express the 3x3 valid conv.

    For output group rows ol in [0,4): feature (ol, oj) gets contributions
      from chunk t rows ri (absolute 4t+ri):   w[ri-ol, c-oj]
      from chunk t+1 rows ri2 (absolute 4t+4+ri2): w[4+ri2-ol, c-oj]
    """
    w = np.asarray(conv_w, np.float32)
    CA = np.zeros((CHUNK_PIX, G_FEAT), np.float32)
    CB = np.zeros((2 * IMG, G_FEAT), np.float32)
    for ol in range(G_ROWS):
        for oj in range(OH):
            m = ol * OH + oj
            for di in range(KW):
                for dj in range(KW):
                    r = ol + di          # row within the 4-row window
                    c = oj + dj
                    if r < CHUNK_ROWS:
                        CA[r * IMG + c, m] = w[di, dj]
                    else:
                        CB[(r - CHUNK_ROWS) * IMG + c, m] = w[di, dj]
    return CA, CB


def _repack_copies():
    """(t, src_lo, src_hi, k, dst_lo) segments mapping conv-group features
    into 128-partition fc tiles: global feature 104t+i -> tile k=(f//128),
    partition f%128."""
    segs = []
    for t in range(N_GROUPS):
        nf = _feat_count(t)
        f0 = G_FEAT * t
        i = 0
        while i < nf:
            k, p = divmod(f0 + i, FC_K)
            n = min(nf - i, FC_K - p)
            segs.append((t, i, i + n, k, p))
            i += n
    return segs


REPACK = _repack_copies()
# Engine per repack copy (sizes 104,24,80,48,56,72,32,96,8,104,16,36):
# balance ~equal elems over ACT ('s', 336 rows) and DVE ('v', 340 rows);
# GPSIMD cannot read PSUM.
COPY_ENG = ['s', 'v', 'v', 's', 'v', 's', 'v', 's', 'v', 'v', 's', 'v']
assert len(COPY_ENG) == len(REPACK)


def build_program():
    f32 = mybir.dt.float32
    st_dt = mybir.dt.bfloat16

    # Bacc (not raw Bass): its compile pipeline splits multi-sem waits into
    # event semaphores — TRN2 instructions carry at most ONE sync wait.
    nc = bacc.Bacc()
    xTs = nc.declare_dram_parameter(
        "xTs", [CHUNK_PIX, N_BLOCKS, N_CHUNKS, NB], st_dt, isOutput=False
    )
    ca_d = nc.declare_dram_parameter("CA", [CHUNK_PIX, G_FEAT], st_dt, isOutput=False)
    cb_d = nc.declare_dram_parameter("CB", [2 * IMG, G_FEAT], st_dt, isOutput=False)
    wp_d = nc.declare_dram_parameter("Wp", [G_FEAT, N_GROUPS, 32], st_dt, isOutput=False)
    bias_d = nc.declare_dram_parameter("bias", [NOUT, 1], f32, isOutput=False)
    outT = nc.declare_dram_parameter("outT", [NOUT, B_CORE], f32, isOutput=True)

    Relu = mybir.ActivationFunctionType.Relu

    with tile.TileContext(nc) as tc:
        with (
            tc.tile_pool(name="const", bufs=1) as const,
            tc.tile_pool(name="x", bufs=6) as xpool,
            tc.tile_pool(name="feat", bufs=3 * N_GROUPS) as fpool,
            tc.tile_pool(name="osb", bufs=4) as opool,
            tc.tile_pool(name="psc", bufs=7, space="PSUM") as psc,
            tc.tile_pool(name="pso", bufs=1, space="PSUM") as pso,
        ):
            # Constants go on the ACT HWDGE ring so they land immediately,
            # in parallel with the x-block loads on the SP/DVE rings.
            ca_sb = const.tile([CHUNK_PIX, G_FEAT], st_dt)
            nc.scalar.dma_start(out=ca_sb[:, :], in_=ca_d[:, :])
            cb_sb = const.tile([2 * IMG, G_FEAT], st_dt)
            nc.scalar.dma_start(out=cb_sb[:, :], in_=cb_d[:, :])
            wp_sb = const.tile([G_FEAT, N_GROUPS, 32], st_dt)
            nc.scalar.dma_start(out=wp_sb[:, :, :], in_=wp_d[:, :, :])
            bias_sb = const.tile([NOUT, 1], f32)
            nc.scalar.dma_start(out=bias_sb[:, :], in_=bias_d[:, :])

            # Short PE warm-up (starts the HAM activity clock while the first
            # x chunk is in flight). CA serves as both operands.
            warm_ps = psc.tile([G_FEAT, NB], mybir.dt.float32, tag="convps")
            for _ in range(12):
                nc.tensor.matmul(
                    warm_ps[:, :G_FEAT], ca_sb[:, :], ca_sb[:, :],
                    start=True, stop=True,
                )

            def emit_block(xt, j):
                # A-phase: all 7 groups against CA (one stationary weight
                # set, gapless PE stream).
                pss = []
                for t in range(N_GROUPS):
                    nf = _feat_count(t)
                    ps = psc.tile([nf, NB], mybir.dt.float32, tag="convps")
                    nc.tensor.matmul(
                        ps[:, :], ca_sb[:, :nf], xt[:, t, :],
                        start=True, stop=(t == N_GROUPS - 1),
                    )
                    pss.append(ps)
                # B-phase: close groups 0..5 against CB.
                for t in range(N_GROUPS - 1):
                    nc.tensor.matmul(
                        pss[t][:, :], cb_sb[:, :],
                        xt[: 2 * IMG, t + 1, :],
                        start=False, stop=True,
                    )
                # ReLU PSUM->SBUF (engine partition lanes are hardwired, so
                # copies keep partition offsets; 3 on ACT, 4 on DVE).
                feats = []
                for t in range(N_GROUPS):
                    nf = _feat_count(t)
                    ft = fpool.tile([nf, NB], st_dt, tag="feat")
                    if t in (0, 2, 4):
                        nc.scalar.activation(ft[:, :], pss[t][:, :], Relu)
                    else:
                        nc.vector.tensor_scalar_max(ft[:, :], pss[t][:, :], 0.0)
                    feats.append(ft)

                # fc: 7 accumulating PE matmuls (tiny 10-col LDWs).
                ops = pso.tile([NOUT, NB], mybir.dt.float32, tag="outps")
                for t in range(N_GROUPS):
                    nf = _feat_count(t)
                    nc.tensor.matmul(
                        ops[:, :], wp_sb[:nf, t, :NOUT], feats[t][:nf, :],
                        start=(t == 0), stop=(t == N_GROUPS - 1),
                    )
                osb = opool.tile([NOUT, NB], f32, tag="osb")
                nc.vector.tensor_scalar(
                    osb[:, :], ops[:, :], bias_sb[:, :], None,
                    op0=mybir.AluOpType.add,
                )
                nc.scalar.dma_start(out=outT[:, ts(j, NB)], in_=osb[:, :])

            for j in range(N_BLOCKS):
                xt = xpool.tile([CHUNK_PIX, N_CHUNKS, NB], st_dt, tag="x")
                ring = nc.sync if j % 2 == 0 else nc.gpsimd
                if j == 0:
                    # Per-chunk DMAs: the A-phase matmul on chunk t can start
                    # as soon as chunk t lands (subtile deps).
                    for t in range(N_CHUNKS):
                        ring.dma_start(out=xt[:, t, :], in_=xTs[:, j, t, :])
                else:
                    ring.dma_start(out=xt[:, :, :], in_=xTs[:, j, :, :])
                emit_block(xt, j)

    nc.finalize()  # runs Bacc.compile(): wait-splitting + register allocation
    return nc


def _np_dt():
    import ml_dtypes

    return ml_dtypes.bfloat16


def prepare_inputs(x, conv_w, W, b):
    dt = _np_dt()
    xf = np.asarray(x, np.float32)
    CA, CB = build_conv_mats(conv_w)
    Wf = np.asarray(W, np.float32)
    Wp = np.zeros((G_FEAT, N_GROUPS, 32), np.float32)
    for t in range(N_GROUPS):
        nf = _feat_count(t)
        Wp[:nf, t, :NOUT] = Wf[G_FEAT * t : G_FEAT * t + nf, :]
    bias = np.asarray(b, np.float32).reshape(NOUT, 1)
    CA, CB, Wp = CA.astype(dt), CB.astype(dt), Wp.astype(dt)
    in_maps = []
    for c in range(N_CORES):
        xc = xf[c * B_CORE : (c + 1) * B_CORE]  # [8192, 784]
        # [p, block, chunk, batch]: per (p, block) the [chunk, batch] plane is
        # contiguous -> 7KB DMA lines.
        xts = (
            xc.reshape(N_BLOCKS, NB, N_CHUNKS, CHUNK_PIX)
            .transpose(3, 0, 2, 1)
            .astype(dt)
        )
        in_maps.append(
            {
                "xTs": np.ascontiguousarray(xts),
                "CA": CA,
                "CB": CB,
                "Wp": Wp,
                "bias": bias,
            }
        )
    return in_maps


def run(x, conv_w, W, b, trace=False, **spmd_kwargs):
    in_maps = prepare_inputs(x, conv_w, W, b)
    nc = build_program()
    res = run_bass_kernel_spmd(
        nc, in_maps, list(range(N_CORES)), trace=trace, **spmd_kwargs
    )
    out = np.empty((B_FULL, NOUT), np.float32)
    for c in range(N_CORES):
        out[c * B_CORE : (c + 1) * B_CORE, :] = res.results[c]["outT"].T
    return out, res


def kernel(x, conv_w, W, b):
    out, _ = run(x, conv_w, W, b, trace=False)
    return out


# revision 15
# speedup vs baseline: 1.7220x; 1.7220x over previous
"""Trainium2 Bass kernel for DigitConvolutionalModel.

Computes: out = relu(conv2d_valid(x.reshape(B,28,28), w3x3)).reshape(B,676) @ W + b

Strategy (pure data parallel over 8 NeuronCores, 8192 images/core):
  - Host: swizzle x per core to xTs[112, 16 blocks, 7 chunks, 512 batch] bf16 so
    each block's DMA reads one 7KB-contiguous segment per partition; build two
    small constant matrices CA[112,104], CB[56,104] that express the 3x3 valid
    conv as matmuls over 4-image-row input chunks; pack W into 6 groups of 128
    features.
  - Device per core: stream 512-image blocks. Each block:
      x DMA alternates between the SP and DVE HWDGE rings (2x queue bw); the
      first block is split into per-chunk DMAs so the conv starts ~4us earlier.
      conv: 7 output groups; groups 0..5 = 2 accumulating PE matmuls
        (CA against chunk t, CB against first 2 rows of chunk t+1), group 6 =
        1 matmul. PSUM [104, 512] x7.
      ReLU PSUM->SBUF repacks the 676 features into 6 tiles of 128 partitions
        (12 partition-shifted copies spread over ACT/DVE/GPSIMD), so the fc
        needs only 6 accumulating matmuls (vs 7 for 104-feature groups).
      fc: 6 accumulating PE matmuls -> PSUM [10, 512]; bias-add via DVE;
        1 DMA store of outT [10, 512] on the GPSIMD ring.
  - Host: gather per-core outT [10, 8192] and transpose into out[B, 10].

Matmul dtype: bfloat16 (fp32 PSUM accumulation). Total HBM read per core is
12.9 MB; PE work is 19 passes x 512 cols x 16 blocks ~= 65us at 2.4 GHz.
"""

import os

import numpy as np

import concourse.bass as bass
import concourse.mybir as mybir
import concourse.tile as tile
from concourse import bacc
from concourse.bass import ts
from concourse.bass_utils import run_bass_kernel_spmd

# Problem geometry (fixed by the task spec)
B_FULL = 65536
IMG = 28
KW = 3
OH = IMG - KW + 1          # 26
NPIX = IMG * IMG           # 784
NFEAT = OH * OH            # 676
NOUT = 10

N_CORES = 8
B_CORE = B_FULL // N_CORES  # 8192
NB = 512                    # images per block
N_BLOCKS = B_CORE // NB     # 16

# Conv grouping: output rows in groups of 4 -> 104 features per group
G_ROWS = 4
G_FEAT = G_ROWS * OH        # 104
N_GROUPS = 7                # 6 full groups of 4 out-rows + 1 group of 2 (52 feats)
CHUNK_ROWS = 4              # input rows per SBUF partition chunk
CHUNK_PIX = CHUNK_ROWS * IMG  # 112
N_CHUNKS = NPIX // CHUNK_PIX  # 7

# fc grouping: features packed 128 per group
FC_K = 128
N_FC = (NFEAT + FC_K - 1) // FC_K  # 6 (5 x 128 + 36)

PRECISION = os.environ.get("KERNEL_PRECISION", "bf16")  # bf16 only


def _feat_count(t: int) -> int:
    # conv group t covers output rows [4t, min(4t+4, 26)) -> features [104t, ..)
    return min(G_FEAT, NFEAT - G_FEAT * t)


def _fc_count(k: int) -> int:
    return min(FC_K, NFEAT - FC_K * k)


def build_conv_mats(conv_w: np.ndarray):
    """CA[(ri,c),(ol,oj)] and CB[(ri2,c),(ol,oj)] express the 3x3 valid conv.

    For output group rows ol in [0,4): feature (ol, oj) gets contributions
      from chunk t rows ri (absolute 4t+ri):   w[ri-ol, c-oj]
      from chunk t+1 rows ri2 (absolute 4t+4+ri2): w[4+ri2-ol, c-oj]
    """
    w = np.asarray(conv_w, np.float32)
    CA = np.zeros((CHUNK_PIX, G_FEAT), np.float32)
    CB = np.zeros((2 * IMG, G_FEAT), np.float32)
    for ol in range(G_ROWS):
        for oj in range(OH):
            m = ol * OH + oj
            for di in range(KW):
                for dj in range(KW):
                    r = ol + di          # row within the 4-row window
                    c = oj + dj
                    if r < CHUNK_ROWS:
                        CA[r * IMG + c, m] = w[di, dj]
                    else:
                        CB[(r - CHUNK_ROWS) * IMG + c, m] = w[di, dj]
    return CA, CB


def _repack_copies():
    """(t, src_lo, src_hi, k, dst_lo) segments mapping conv-group features
    into 128-partition fc tiles: global feature 104t+i -> tile k=(f//128),
    partition f%128."""
    segs = []
    for t in range(N_GROUPS):
        nf = _feat_count(t)
        f0 = G_FEAT * t
        i = 0
        while i < nf:
            k, p = divmod(f0 + i, FC_K)
            n = min(nf - i, FC_K - p)
            segs.append((t, i, i + n, k, p))
            i += n
    return segs


REPACK = _repack_copies()
# Engine per repack copy (sizes 104,24,80,48,56,72,32,96,8,104,16,36):
# balance ~equal elems over ACT ('s', 336 rows) and DVE ('v', 340 rows);
# GPSIMD cannot read PSUM.
COPY_ENG = ['s', 'v', 'v', 's', 'v', 's', 'v', 's', 'v', 'v', 's', 'v']
assert len(COPY_ENG) == len(REPACK)


def build_program():
    f32 = mybir.dt.float32
    st_dt = mybir.dt.bfloat16

    # Bacc (not raw Bass): its compile pipeline splits multi-sem waits into
    # event semaphores — TRN2 instructions carry at most ONE sync wait.
    nc = bacc.Bacc()
    xTs = nc.declare_dram_parameter(
        "xTs", [CHUNK_PIX, N_BLOCKS, N_CHUNKS, NB], st_dt, isOutput=False
    )
    ca_d = nc.declare_dram_parameter("CA", [CHUNK_PIX, G_FEAT], st_dt, isOutput=False)
    cb_d = nc.declare_dram_parameter("CB", [2 * IMG, G_FEAT], st_dt, isOutput=False)
    wp_d = nc.declare_dram_parameter("Wp", [G_FEAT, N_GROUPS, 32], st_dt, isOutput=False)
    bias_d = nc.declare_dram_parameter("bias", [NOUT, 1], f32, isOutput=False)
    outT = nc.declare_dram_parameter("outT", [NOUT, B_CORE], f32, isOutput=True)

    Relu = mybir.ActivationFunctionType.Relu

    with tile.TileContext(nc) as tc:
        with (
            tc.tile_pool(name="const", bufs=1) as const,
            tc.tile_pool(name="x", bufs=6) as xpool,
            tc.tile_pool(name="feat", bufs=3 * N_GROUPS) as fpool,
            tc.tile_pool(name="osb", bufs=4) as opool,
            tc.tile_pool(name="psc", bufs=7, space="PSUM") as psc,
            tc.tile_pool(name="pso", bufs=1, space="PSUM") as pso,
        ):
            # Constants go on the ACT HWDGE ring so they land immediately,
            # in parallel with the x-block loads on the SP/DVE rings.
            ca_sb = const.tile([CHUNK_PIX, G_FEAT], st_dt)
            nc.scalar.dma_start(out=ca_sb[:, :], in_=ca_d[:, :])
            cb_sb = const.tile([2 * IMG, G_FEAT], st_dt)
            nc.scalar.dma_start(out=cb_sb[:, :], in_=cb_d[:, :])
            wp_sb = const.tile([G_FEAT, N_GROUPS, 32], st_dt)
            nc.scalar.dma_start(out=wp_sb[:, :, :], in_=wp_d[:, :, :])
            bias_sb = const.tile([NOUT, 1], f32)
            nc.scalar.dma_start(out=bias_sb[:, :], in_=bias_d[:, :])

            # Short PE warm-up (starts the HAM activity clock while the first
            # x chunk is in flight). CA serves as both operands.
            warm_ps = psc.tile([G_FEAT, NB], mybir.dt.float32, tag="convps")
            for _ in range(32):
                nc.tensor.matmul(
                    warm_ps[:, :G_FEAT], ca_sb[:, :], ca_sb[:, :],
                    start=True, stop=True,
                )

            def emit_block(xt, j):
                # A-phase: all 7 groups against CA (one stationary weight
                # set, gapless PE stream).
                pss = []
                for t in range(N_GROUPS):
                    nf = _feat_count(t)
                    ps = psc.tile([nf, NB], mybir.dt.float32, tag="convps")
                    nc.tensor.matmul(
                        ps[:, :], ca_sb[:, :nf], xt[:, t, :],
                        start=True, stop=(t == N_GROUPS - 1),
                    )
                    pss.append(ps)
                # B-phase: close groups 0..5 against CB.
                for t in range(N_GROUPS - 1):
                    nc.tensor.matmul(
                        pss[t][:, :], cb_sb[:, :],
                        xt[: 2 * IMG, t + 1, :],
                        start=False, stop=True,
                    )
                # ReLU PSUM->SBUF (engine partition lanes are hardwired, so
                # copies keep partition offsets; 3 on ACT, 4 on DVE).
                feats = []
                for t in range(N_GROUPS):
                    nf = _feat_count(t)
                    ft = fpool.tile([nf, NB], st_dt, tag="feat")
                    if t % 2 == 0:
                        nc.scalar.activation(ft[:, :], pss[t][:, :], Relu)
                    else:
                        nc.vector.tensor_scalar_max(ft[:, :], pss[t][:, :], 0.0)
                    feats.append(ft)

                # fc: 7 accumulating PE matmuls (tiny 10-col LDWs).
                ops = pso.tile([NOUT, NB], mybir.dt.float32, tag="outps")
                for t in range(N_GROUPS):
                    nf = _feat_count(t)
                    nc.tensor.matmul(
                        ops[:, :], wp_sb[:nf, t, :NOUT], feats[t][:nf, :],
                        start=(t == 0), stop=(t == N_GROUPS - 1),
                    )
                osb = opool.tile([NOUT, NB], f32, tag="osb")
                nc.vector.tensor_scalar(
                    osb[:, :], ops[:, :], bias_sb[:, :], None,
                    op0=mybir.AluOpType.add,
                )
                nc.scalar.dma_start(out=outT[:, ts(j, NB)], in_=osb[:, :])

            for j in range(N_BLOCKS):
                xt = xpool.tile([CHUNK_PIX, N_CHUNKS, NB], st_dt, tag="x")
                ring = nc.sync
                if j == 0:
                    # Per-chunk DMAs: the A-phase matmul on chunk t can start
                    # as soon as chunk t lands (subtile deps).
                    for t in range(N_CHUNKS):
                        ring.dma_start(out=xt[:, t, :], in_=xTs[:, j, t, :])
                else:
                    ring.dma_start(out=xt[:, :, :], in_=xTs[:, j, :, :])
                emit_block(xt, j)

    nc.finalize()  # runs Bacc.compile(): wait-splitting + register allocation
    return nc


def _np_dt():
    import ml_dtypes

    return ml_dtypes.bfloat16


def prepare_inputs(x, conv_w, W, b):
    dt = _np_dt()
    xf = np.asarray(x, np.float32)
    CA, CB = build_conv_mats(conv_w)
    Wf = np.asarray(W, np.float32)
    Wp = np.zeros((G_FEAT, N_GROUPS, 32), np.float32)
    for t in range(N_GROUPS):
        nf = _feat_count(t)
        Wp[:nf, t, :NOUT] = Wf[G_FEAT * t : G_FEAT * t + nf, :]
    bias = np.asarray(b, np.float32).reshape(NOUT, 1)
    CA, CB, Wp = CA.astype(dt), CB.astype(dt), Wp.astype(dt)
    in_maps = []
    for c in range(N_CORES):
        xc = xf[c * B_CORE : (c + 1) * B_CORE]  # [8192, 784]
        # [p, block, chunk, batch]: per (p, block) the [chunk, batch] plane is
        # contiguous -> 7KB DMA lines.
        xts = (
            xc.reshape(N_BLOCKS, NB, N_CHUNKS, CHUNK_PIX)
            .transpose(3, 0, 2, 1)
            .astype(dt)
        )
        in_maps.append(
            {
                "xTs": np.ascontiguousarray(xts),
                "CA": CA,
                "CB": CB,
                "Wp": Wp,
                "bias": bias,
            }
        )
    return in_maps


def run(x, conv_w, W, b, trace=False, **spmd_kwargs):
    in_maps = prepare_inputs(x, conv_w, W, b)
    nc = build_program()
    res = run_bass_kernel_spmd(
        nc, in_maps, list(range(N_CORES)), trace=trace, **spmd_kwargs
    )
    out = np.empty((B_FULL, NOUT), np.float32)
    for c in range(N_CORES):
        out[c * B_CORE : (c + 1) * B_CORE, :] = res.results[c]["outT"].T
    return out, res


def kernel(x, conv_w, W, b):
    out, _ = run(x, conv_w, W, b, trace=False)
    return out
